# revision 25
# baseline (speedup 1.0000x reference)
"""Tensor-parallel causal attention block for 8 Trainium2 NeuronCores.

Sharding: 2-way batch data-parallel x 4-way head tensor-parallel.  Each core
handles one batch's tokens for 4 of the 16 heads: q/k/v projections (columns
of wq/wk/wv), RoPE, causal attention, and a row-slice of the output
projection (rows of wo).  The host sums the 4 partial outputs per batch.

All matmul operands are fp16 (fp32 PSUM accumulation): same PE streaming
rate as fp32r (1 cycle/row at N>=256) but half the DMA/SBUF traffic and
2-4x DVE throughput.  V is projected directly in natural [token, dim]
layout using x-tiles as the stationary operand, eliminating all PE
transposes.  Scores are computed transposed (S^T[k, q]) so softmax
renormalization folds into PE ones-matmuls.  The softmax reciprocal runs
on the scalar engine (ACT Reciprocal), not the slow DVE reciprocal.
RoPE half-swaps are triggered from the gpsimd queue to keep the sync DMA
queue free for the x/weight stream.
"""

import math
import sys

sys.path.insert(0, "/opt/trn_rl_repo")

import numpy as np

B = 2
S = 2048
E = 2048
H = 16
D = 128
ROPE_BASE = 10000.0
NCORES = 8
BGRP = 2                   # batch groups
HPC = H // (NCORES // BGRP)  # heads per core = 4
DC = HPC * D               # head-dim cols per core = 512
KC = E // 128              # 16 contraction chunks
TC = 512                   # token chunk for projections
NCH = S // TC              # 4 chunks
NSB = S // 512             # 4 query superblocks
SCALE = 1.0 / math.sqrt(D)
LOOKAHEAD = 4   # key-block tiles of score lead over the z/sum consumers

_COMPILED = None


def _build_program():
    import concourse.bass as bass
    import concourse.mybir as mybir
    from concourse import bacc
    from concourse.tile import TileContext

    f32 = mybir.dt.float32
    f16 = mybir.dt.float16

    nc = bacc.Bacc()
    # host-blocked layouts: every DMA tile is contiguous in DRAM
    xT_d = nc.declare_dram_parameter("xT", [KC, NCH, 128, TC], f16, isOutput=False)
    cos_d = nc.declare_dram_parameter("cosF", [128, S], f16, isOutput=False)
    sin_d = nc.declare_dram_parameter("sinF", [128, S], f16, isOutput=False)
    wq_d = nc.declare_dram_parameter("wq", [KC, 128, DC], f16, isOutput=False)
    wk_d = nc.declare_dram_parameter("wk", [KC, 128, DC], f16, isOutput=False)
    wv_d = nc.declare_dram_parameter("wv", [KC, 128, DC], f16, isOutput=False)
    wo_d = nc.declare_dram_parameter("wo", [128, HPC, E], f16, isOutput=False)
    on_d = nc.declare_dram_parameter("ones", [128, 128], f16, isOutput=False)
    out_d = nc.declare_dram_parameter("out", [S // 128, 128, E], f16, isOutput=True)

    Exp = mybir.ActivationFunctionType.Exp
    mult = mybir.AluOpType.mult
    add = mybir.AluOpType.add

    with TileContext(nc) as tc:
        with (
            tc.tile_pool(name="wpool", bufs=1) as wp,
            tc.tile_pool(name="persist", bufs=1) as pp,
            tc.tile_pool(name="xin", bufs=16) as xp,
            tc.tile_pool(name="rope", bufs=3) as rp,
            tc.tile_pool(name="ptile", bufs=6) as ptp,
            tc.tile_pool(name="small", bufs=2) as smp,
            tc.tile_pool(name="outsb", bufs=2) as op,
        ):
            # ---- resident weights / constants (DMAs stream inside chunk 0) ----
            wq_sb = wp.tile([128, KC, DC], f16)
            wk_sb = wp.tile([128, KC, DC], f16)
            wv_sb = wp.tile([128, KC, DC], f16)
            wo_sb = wp.tile([128, HPC, E], f16)
            cos_sb = wp.tile([128, S], f16)
            sin_sb = wp.tile([128, S], f16)
            ones_sb = wp.tile([128, 128], f16)

            # ---- persistent arrays ----
            qT = [pp.tile([128, S], f16, name=f"qT{h}", tag=f"qT{h}") for h in range(HPC)]
            kT = [pp.tile([128, S], f16, name=f"kT{h}", tag=f"kT{h}") for h in range(HPC)]
            v_sb = pp.tile([128, S // 128, DC], f16, name="v_sb", tag="v")
            zn = [pp.tile([128, S], f16, name=f"zn{h}", tag=f"zn{h}") for h in range(HPC)]

            xts = [[None] * (KC // 2) for _ in range(NCH)]

            def fetch_x(c):
                # paired DMAs: one trigger per 2 contraction chunks
                for k2 in range(KC // 2):
                    xt = xp.tile([128, 2, TC], f16, name="xt")
                    nc.sync.dma_start(
                        out=xt[:], in_=xT_d[2 * k2:2 * k2 + 2, c].rearrange("a p c -> p a c"))
                    xts[c][k2] = xt

            def xtile(c, kc, j0=0, j1=TC):
                return xts[c][kc // 2][:, kc % 2, j0:j1]

            def rope_drain(ps, dst, s0):
                # tmp = raw q/k (fp16), rot = half-swapped copy; out = tmp*cos + rot*sin
                tmp = rp.tile([128, TC], f16, name="tmp", tag="tmp")
                nc.scalar.copy(tmp[:], ps[:])
                rot = rp.tile([128, TC], f16, name="rot", tag="rot")
                nc.gpsimd.dma_start(out=rot[0:64, :], in_=tmp[64:128, :])
                nc.gpsimd.dma_start(out=rot[64:128, :], in_=tmp[0:64, :])
                nc.vector.tensor_tensor(tmp[:], tmp[:], cos_sb[:, s0:s0 + TC], mult)
                nc.vector.tensor_tensor(rot[:], rot[:], sin_sb[:, s0:s0 + TC], mult)
                nc.vector.tensor_tensor(dst[:, s0:s0 + TC], tmp[:], rot[:], add)

            # ============ Phase A: projections + RoPE ============
            with tc.tile_pool(name="psA", bufs=1, space="PSUM") as pA:
                for c in range(NCH):
                    s0 = c * TC
                    q_ps = [None] * HPC
                    k_ps = [None] * HPC
                    if c == 0:
                        # HAM warm-up: dummy matmuls on memset scratch keep the
                        # PE clock-gate busy while the first weights stream in
                        warm = wp.tile([128, TC], f16, name="warm")
                        nc.gpsimd.memset(warm[:], 0)
                        wm_ps = pA.tile([128, TC], f32, name="wm_ps", tag="t7")
                        for _ in range(12):
                            nc.tensor.matmul(wm_ps[:], lhsT=warm[:, 0:128], rhs=warm[:],
                                             start=True, stop=True)
                        # kc-outer so the weight stream overlaps the matmuls;
                        # wv/cos/sin/ones go on the gpsimd queue so the sync
                        # queue carries only the wq/wk/x stream
                        for h in range(HPC):
                            q_ps[h] = pA.tile([128, TC], f32, name=f"q_ps{h}", tag=f"t{2 * h}")
                            k_ps[h] = pA.tile([128, TC], f32, name=f"k_ps{h}", tag=f"t{2 * h + 1}")
                        for kc in range(KC):
                            if kc % 2 == 0:
                                xt = xp.tile([128, 2, TC], f16, name="xt")
                                nc.sync.dma_start(
                                    out=xt[:], in_=xT_d[kc:kc + 2, 0].rearrange("a p c -> p a c"))
                                xts[0][kc // 2] = xt
                            if kc % 4 == 0:
                                nc.sync.dma_start(out=wq_sb[:, kc:kc + 4, :],
                                                  in_=wq_d[kc:kc + 4].rearrange("a p c -> p a c"))
                                nc.sync.dma_start(out=wk_sb[:, kc:kc + 4, :],
                                                  in_=wk_d[kc:kc + 4].rearrange("a p c -> p a c"))
                            if kc == 6:
                                nc.gpsimd.dma_start(out=cos_sb[:], in_=cos_d[:])
                                nc.gpsimd.dma_start(out=sin_sb[:], in_=sin_d[:])
                                nc.gpsimd.dma_start(out=ones_sb[:], in_=on_d[:])
                            if kc >= 8:  # wv needed at part 2 of this chunk
                                k2 = (kc - 8) * 2
                                nc.gpsimd.dma_start(out=wv_sb[:, k2:k2 + 2, :],
                                                    in_=wv_d[k2:k2 + 2].rearrange("a p c -> p a c"))
                            for h in range(HPC):
                                nc.tensor.matmul(q_ps[h][:], lhsT=wq_sb[:, kc, h * D:(h + 1) * D],
                                                 rhs=xtile(0, kc), start=(kc == 0), stop=(kc == KC - 1))
                                nc.tensor.matmul(k_ps[h][:], lhsT=wk_sb[:, kc, h * D:(h + 1) * D],
                                                 rhs=xtile(0, kc), start=(kc == 0), stop=(kc == KC - 1))
                        for h in range(HPC):
                            rope_drain(q_ps[h], qT[h], s0)
                            rope_drain(k_ps[h], kT[h], s0)
                    else:
                        # weights resident: h-outer so each psum drains while
                        # the next head's matmuls run
                        if c == 2:
                            nc.gpsimd.dma_start(out=wo_sb[:], in_=wo_d[:])
                        for h in range(HPC):
                            q_ps[h] = pA.tile([128, TC], f32, name=f"q_ps{h}", tag=f"t{2 * h}")
                            for kc in range(KC):
                                nc.tensor.matmul(q_ps[h][:], lhsT=wq_sb[:, kc, h * D:(h + 1) * D],
                                                 rhs=xtile(c, kc), start=(kc == 0), stop=(kc == KC - 1))
                            rope_drain(q_ps[h], qT[h], s0)
                            k_ps[h] = pA.tile([128, TC], f32, name=f"k_ps{h}", tag=f"t{2 * h + 1}")
                            for kc in range(KC):
                                nc.tensor.matmul(k_ps[h][:], lhsT=wk_sb[:, kc, h * D:(h + 1) * D],
                                                 rhs=xtile(c, kc), start=(kc == 0), stop=(kc == KC - 1))
                            rope_drain(k_ps[h], kT[h], s0)
                    # part 2: V in natural layout (x-tile stationary); the
                    # next chunk's x stream is queued first so it prefetches
                    # during this chunk's remaining compute
                    if c + 1 < NCH:
                        fetch_x(c + 1)
                    for tb in range(TC // 128):
                        v_ps = pA.tile([128, DC], f32, name="v_ps", tag=f"t{2 * tb}")
                        for kc in range(KC):
                            nc.tensor.matmul(v_ps[:], lhsT=xtile(c, kc, tb * 128, (tb + 1) * 128),
                                             rhs=wv_sb[:, kc, :], start=(kc == 0), stop=(kc == KC - 1))
                        nc.vector.tensor_copy(v_sb[:, c * (TC // 128) + tb, :], v_ps[:])

            # ============ Phase B: causal attention (flat pipeline) ============
            # Score matmuls lead z/sum matmuls by a global LOOKAHEAD window
            # that flows ACROSS (head, superblock) unit boundaries, so the
            # exp pipeline never drains.  Phase C token-block groups are
            # interleaved after each superblock (they only need that
            # superblock's zn) and share the "st" PSUM tag, filling the
            # attention pipeline's dependency-wait slack with matmuls.
            with tc.tile_pool(name="psB", bufs=1, space="PSUM") as pB:
                tiles = []   # (sb, h, kblk)
                cpoint = {}
                for sb in range(NSB):
                    for h in range(HPC):
                        for kblk in range((sb + 1) * 4):
                            tiles.append((sb, h, kblk))
                    cpoint[len(tiles)] = sb
                pts = {}
                zsum = {}

                def emit_score(t):
                    sb, h, kblk = t
                    # diagonal tiles only need queries q >= 128*delta:
                    # restrict score/exp/mask to the live column range
                    delta = kblk - sb * 4
                    q0 = 128 * delta if delta > 0 else 0
                    st_ps = pB.tile([128, 512], f32, name="st_ps", tag="st", bufs=4)
                    nc.tensor.matmul(st_ps[:, q0:512],
                                     lhsT=kT[h][:, kblk * 128:(kblk + 1) * 128],
                                     rhs=qT[h][:, sb * 512 + q0:(sb + 1) * 512],
                                     start=True, stop=True)
                    pt = ptp.tile([128, 512], f16, name="pt", tag="pt")
                    nc.scalar.activation(pt[:, q0:512], st_ps[:, q0:512], Exp, scale=SCALE)
                    if delta >= 0:
                        nc.gpsimd.affine_select(
                            out=pt[:, q0:q0 + 128], in_=pt[:, q0:q0 + 128],
                            pattern=[[1, 128]], compare_op=mybir.AluOpType.is_ge,
                            fill=0.0, base=0, channel_multiplier=-1,
                        )
                    pts[t] = (pt, q0)

                def emit_zsum(t):
                    sb, h, kblk = t
                    nkb = (sb + 1) * 4
                    if kblk == 0:
                        zsum[(sb, h)] = (
                            pB.tile([128, 512], f32, name="z_ps", tag="z", bufs=2),
                            pB.tile([128, 512], f32, name="sum_ps", tag="sum", bufs=2),
                        )
                    z_ps, sum_ps = zsum[(sb, h)]
                    pt, q0 = pts.pop(t)
                    nc.tensor.matmul(z_ps[:, q0:512], lhsT=v_sb[:, kblk, h * D:(h + 1) * D],
                                     rhs=pt[:, q0:512], start=(kblk == 0), stop=(kblk == nkb - 1))
                    nc.tensor.matmul(sum_ps[:, q0:512], lhsT=ones_sb[:],
                                     rhs=pt[:, q0:512], start=(kblk == 0), stop=(kblk == nkb - 1))
                    if kblk == nkb - 1:
                        rep_sb = smp.tile([128, 512], f32, name="rep_sb", tag="repsb")
                        nc.vector.reciprocal_approx_fast(rep_sb[:], sum_ps[:])
                        nc.vector.tensor_tensor(zn[h][:, sb * 512:(sb + 1) * 512],
                                                z_ps[:], rep_sb[:], mult)

                def emit_ctb(tb):
                    # output projection for one 128-token block: independent
                    # PE work that absorbs the attention pipeline's exp-wait
                    o_sb = op.tile([128, E], f16, name="o_sb", tag="osb")
                    for ec in range(E // 512):
                        o_ps = pB.tile([128, 512], f32, name="o_ps", tag="st", bufs=4)
                        for h in range(HPC):
                            nc.tensor.matmul(o_ps[:], lhsT=zn[h][:, tb * 128:(tb + 1) * 128],
                                             rhs=wo_sb[:, h, ec * 512:(ec + 1) * 512],
                                             start=(h == 0), stop=(h == HPC - 1))
                        if ec % 2 == 0:
                            nc.vector.tensor_copy(o_sb[:, ec * 512:(ec + 1) * 512], o_ps[:])
                        else:
                            nc.scalar.copy(o_sb[:, ec * 512:(ec + 1) * 512], o_ps[:])
                    nc.sync.dma_start(out=out_d[tb], in_=o_sb[:])

                for j in range(min(LOOKAHEAD, len(tiles))):
                    emit_score(tiles[j])
                for j, t in enumerate(tiles):
                    if j + LOOKAHEAD < len(tiles):
                        emit_score(tiles[j + LOOKAHEAD])
                    emit_zsum(t)
                    if (j + 1) in cpoint:
                        sb = cpoint[j + 1]
                        for tb in range(sb * 4, (sb + 1) * 4):
                            emit_ctb(tb)

    nc.compile()
    return nc


def _get_compiled():
    global _COMPILED
    if _COMPILED is None:
        _COMPILED = _build_program()
    return _COMPILED


def _host_inputs(x, wq, wk, wv, wo):
    x = np.asarray(x, dtype=np.float32)
    # per batch: xT blocked [KC, NCH, 128, TC]; (kc, c, p, t) = x[b, c*TC+t, kc*128+p]
    xTb = []
    for b in range(B):
        xb = np.ascontiguousarray(
            x[b].T.reshape(KC, 128, NCH, TC).transpose(0, 2, 1, 3).astype(np.float16)
        )
        xTb.append(xb)

    pos = np.arange(S, dtype=np.float32)
    inv_freq = (1.0 / (ROPE_BASE ** (np.arange(0, D, 2, dtype=np.float32) / np.float32(D)))).astype(np.float32)
    ang = pos[:, None] * inv_freq[None, :]          # (S, 64) fp32
    cos_h = np.cos(ang)
    sin_h = np.sin(ang)
    cosF = np.ascontiguousarray(np.concatenate([cos_h.T, cos_h.T], axis=0)).astype(np.float16)
    sinF = np.ascontiguousarray(np.concatenate([-sin_h.T, sin_h.T], axis=0)).astype(np.float16)
    ones = np.ones((128, 128), dtype=np.float16)

    wq = np.asarray(wq, dtype=np.float32)
    wk = np.asarray(wk, dtype=np.float32)
    wv = np.asarray(wv, dtype=np.float32)
    wo = np.asarray(wo, dtype=np.float32)

    maps = []
    for core in range(NCORES):
        b = core // (NCORES // BGRP)
        g = core % (NCORES // BGRP)
        sl = slice(g * DC, (g + 1) * DC)
        maps.append({
            "xT": xTb[b],
            "cosF": cosF,
            "sinF": sinF,
            "wq": np.ascontiguousarray(wq[:, sl].reshape(KC, 128, DC)).astype(np.float16),
            "wk": np.ascontiguousarray(wk[:, sl].reshape(KC, 128, DC)).astype(np.float16),
            "wv": np.ascontiguousarray(wv[:, sl].reshape(KC, 128, DC)).astype(np.float16),
            "wo": np.ascontiguousarray(wo[sl, :].reshape(HPC, 128, E).transpose(1, 0, 2)).astype(np.float16),
            "ones": ones,
        })
    return maps


def kernel(x, wq, wk, wv, wo, _trace=False):
    from concourse.bass_utils import run_bass_kernel_spmd

    nc = _get_compiled()
    maps = _host_inputs(x, wq, wk, wv, wo)
    res = run_bass_kernel_spmd(nc, maps, list(range(NCORES)), trace=_trace)
    out = np.zeros((B, S, E), dtype=np.float32)
    for core in range(NCORES):
        b = core // (NCORES // BGRP)
        out[b] += res.results[core]["out"].astype(np.float32).reshape(S, E)
    if _trace:
        kernel.last_exec_time_ns = res.exec_time_ns
        kernel.last_trace = res.instructions_and_trace
    return out


# revision 31
# speedup vs baseline: 1.2013x; 1.2013x over previous
"""Tensor-parallel causal attention block for 8 Trainium2 NeuronCores.

Sharding: 2-way batch data-parallel x 4-way head tensor-parallel.  Each core
handles one batch's tokens for 4 of the 16 heads: q/k/v projections (columns
of wq/wk/wv), RoPE, causal attention, and a row-slice of the output
projection (rows of wo).  The host sums the 4 partial outputs per batch.

All matmul operands are fp16 (fp32 PSUM accumulation): same PE streaming
rate as fp32r (1 cycle/row at N>=256) but half the DMA/SBUF traffic and
2-4x DVE throughput.  V is projected directly in natural [token, dim]
layout using x-tiles as the stationary operand, eliminating all PE
transposes.  Scores are computed transposed (S^T[k, q]) so softmax
renormalization folds into PE ones-matmuls.  The softmax reciprocal runs
on the scalar engine (ACT Reciprocal), not the slow DVE reciprocal.
RoPE half-swaps are triggered from the gpsimd queue to keep the sync DMA
queue free for the x/weight stream.
"""

import math
import sys

sys.path.insert(0, "/opt/trn_rl_repo")

import numpy as np

B = 2
S = 2048
E = 2048
H = 16
D = 128
ROPE_BASE = 10000.0
NCORES = 8
BGRP = 2                   # batch groups
HPC = H // (NCORES // BGRP)  # heads per core = 4
DC = HPC * D               # head-dim cols per core = 512
KC = E // 128              # 16 contraction chunks
TC = 512                   # token chunk for projections
NCH = S // TC              # 4 chunks
NSB = S // 512             # 4 query superblocks
SCALE = 1.0 / math.sqrt(D)
LOOKAHEAD = 4   # key-block tiles of score lead over the z/sum consumers

_COMPILED = None


def _build_program():
    import concourse.bass as bass
    import concourse.mybir as mybir
    from concourse import bacc
    from concourse.tile import TileContext

    f32 = mybir.dt.float32
    f16 = mybir.dt.float16

    nc = bacc.Bacc()
    # host-blocked layouts: every DMA tile is contiguous in DRAM
    xT_d = nc.declare_dram_parameter("xT", [KC, NCH, 128, TC], f16, isOutput=False)
    cos_d = nc.declare_dram_parameter("cosF", [128, S], f16, isOutput=False)
    sin_d = nc.declare_dram_parameter("sinF", [128, S], f16, isOutput=False)
    wq_d = nc.declare_dram_parameter("wq", [KC, 128, DC], f16, isOutput=False)
    wk_d = nc.declare_dram_parameter("wk", [KC, 128, DC], f16, isOutput=False)
    wv_d = nc.declare_dram_parameter("wv", [KC, 128, DC], f16, isOutput=False)
    wo_d = nc.declare_dram_parameter("wo", [128, HPC, E], f16, isOutput=False)
    on_d = nc.declare_dram_parameter("ones", [128, 128], f16, isOutput=False)
    tri_d = nc.declare_dram_parameter("tri", [128, 128], f16, isOutput=False)
    out_d = nc.declare_dram_parameter("out", [S // 128, 128, E], f16, isOutput=True)

    Exp = mybir.ActivationFunctionType.Exp
    mult = mybir.AluOpType.mult
    add = mybir.AluOpType.add

    with TileContext(nc) as tc:
        with (
            tc.tile_pool(name="wpool", bufs=1) as wp,
            tc.tile_pool(name="persist", bufs=1) as pp,
            tc.tile_pool(name="xin", bufs=16) as xp,
            tc.tile_pool(name="rope", bufs=3) as rp,
            tc.tile_pool(name="ptile", bufs=6) as ptp,
            tc.tile_pool(name="small", bufs=2) as smp,
            tc.tile_pool(name="outsb", bufs=2) as op,
        ):
            # ---- resident weights / constants (DMAs stream inside chunk 0) ----
            wq_sb = wp.tile([128, KC, DC], f16)
            wk_sb = wp.tile([128, KC, DC], f16)
            wv_sb = wp.tile([128, KC, DC], f16)
            wo_sb = wp.tile([128, HPC, E], f16)
            cos_sb = wp.tile([128, S], f16)
            sin_sb = wp.tile([128, S], f16)
            ones_sb = wp.tile([128, 128], f16)
            tri_sb = wp.tile([128, 128], f16)

            # ---- persistent arrays ----
            qT = [pp.tile([128, S], f16, name=f"qT{h}", tag=f"qT{h}") for h in range(HPC)]
            kT = [pp.tile([128, S], f16, name=f"kT{h}", tag=f"kT{h}") for h in range(HPC)]
            v_sb = pp.tile([128, S // 128, DC], f16, name="v_sb", tag="v")
            zn = [pp.tile([128, S], f16, name=f"zn{h}", tag=f"zn{h}") for h in range(HPC)]

            xts = [[None] * (KC // 2) for _ in range(NCH)]

            def fetch_x(c):
                # paired DMAs: one trigger per 2 contraction chunks
                for k2 in range(KC // 2):
                    xt = xp.tile([128, 2, TC], f16, name="xt")
                    nc.sync.dma_start(
                        out=xt[:], in_=xT_d[2 * k2:2 * k2 + 2, c].rearrange("a p c -> p a c"))
                    xts[c][k2] = xt

            def xtile(c, kc, j0=0, j1=TC):
                return xts[c][kc // 2][:, kc % 2, j0:j1]

            def rope_drain(ps, dst, s0):
                # tmp = raw q/k (fp16), rot = half-swapped copy; out = tmp*cos + rot*sin
                tmp = rp.tile([128, TC], f16, name="tmp", tag="tmp")
                nc.scalar.copy(tmp[:], ps[:])
                rot = rp.tile([128, TC], f16, name="rot", tag="rot")
                nc.gpsimd.dma_start(out=rot[0:64, :], in_=tmp[64:128, :])
                nc.gpsimd.dma_start(out=rot[64:128, :], in_=tmp[0:64, :])
                nc.vector.tensor_tensor(tmp[:], tmp[:], cos_sb[:, s0:s0 + TC], mult)
                nc.vector.tensor_tensor(rot[:], rot[:], sin_sb[:, s0:s0 + TC], mult)
                nc.vector.tensor_tensor(dst[:, s0:s0 + TC], tmp[:], rot[:], add)

            # ============ Phase A: projections + RoPE ============
            with tc.tile_pool(name="psA", bufs=1, space="PSUM") as pA:
                for c in range(NCH):
                    s0 = c * TC
                    q_ps = [None] * HPC
                    k_ps = [None] * HPC
                    if c == 0:
                        # HAM warm-up: dummy matmuls on memset scratch keep the
                        # PE clock-gate busy while the first weights stream in
                        warm = wp.tile([128, TC], f16, name="warm")
                        nc.gpsimd.memset(warm[:], 0)
                        wm_ps = pA.tile([128, TC], f32, name="wm_ps", tag="t7")
                        for _ in range(12):
                            nc.tensor.matmul(wm_ps[:], lhsT=warm[:, 0:128], rhs=warm[:],
                                             start=True, stop=True)
                        # kc-outer so the weight stream overlaps the matmuls;
                        # wv/cos/sin/ones go on the gpsimd queue so the sync
                        # queue carries only the wq/wk/x stream
                        for h in range(HPC):
                            q_ps[h] = pA.tile([128, TC], f32, name=f"q_ps{h}", tag=f"t{2 * h}")
                            k_ps[h] = pA.tile([128, TC], f32, name=f"k_ps{h}", tag=f"t{2 * h + 1}")
                        for kc in range(KC):
                            if kc % 2 == 0:
                                xt = xp.tile([128, 2, TC], f16, name="xt")
                                nc.sync.dma_start(
                                    out=xt[:], in_=xT_d[kc:kc + 2, 0].rearrange("a p c -> p a c"))
                                xts[0][kc // 2] = xt
                            if kc % 4 == 0:
                                # wq/x on the sync queue, wk on the gpsimd
                                # queue: two trigger streams engage more DMA
                                # engines during the bandwidth-bound chunk 0
                                nc.sync.dma_start(out=wq_sb[:, kc:kc + 4, :],
                                                  in_=wq_d[kc:kc + 4].rearrange("a p c -> p a c"))
                                nc.gpsimd.dma_start(out=wk_sb[:, kc:kc + 4, :],
                                                    in_=wk_d[kc:kc + 4].rearrange("a p c -> p a c"))
                            if kc == 6:
                                nc.gpsimd.dma_start(out=cos_sb[:], in_=cos_d[:])
                                nc.gpsimd.dma_start(out=sin_sb[:], in_=sin_d[:])
                                nc.gpsimd.dma_start(out=ones_sb[:], in_=on_d[:])
                                nc.gpsimd.dma_start(out=tri_sb[:], in_=tri_d[:])
                            if kc >= 8:  # wv needed at part 2 of this chunk
                                k2 = (kc - 8) * 2
                                nc.gpsimd.dma_start(out=wv_sb[:, k2:k2 + 2, :],
                                                    in_=wv_d[k2:k2 + 2].rearrange("a p c -> p a c"))
                            for h in range(HPC):
                                nc.tensor.matmul(q_ps[h][:], lhsT=wq_sb[:, kc, h * D:(h + 1) * D],
                                                 rhs=xtile(0, kc), start=(kc == 0), stop=(kc == KC - 1))
                                nc.tensor.matmul(k_ps[h][:], lhsT=wk_sb[:, kc, h * D:(h + 1) * D],
                                                 rhs=xtile(0, kc), start=(kc == 0), stop=(kc == KC - 1))
                        for h in range(HPC):
                            rope_drain(q_ps[h], qT[h], s0)
                            rope_drain(k_ps[h], kT[h], s0)
                    else:
                        # weights resident: h-outer so each psum drains while
                        # the next head's matmuls run
                        if c == 2:
                            nc.gpsimd.dma_start(out=wo_sb[:], in_=wo_d[:])
                        for h in range(HPC):
                            q_ps[h] = pA.tile([128, TC], f32, name=f"q_ps{h}", tag=f"t{2 * h}")
                            for kc in range(KC):
                                nc.tensor.matmul(q_ps[h][:], lhsT=wq_sb[:, kc, h * D:(h + 1) * D],
                                                 rhs=xtile(c, kc), start=(kc == 0), stop=(kc == KC - 1))
                            rope_drain(q_ps[h], qT[h], s0)
                            k_ps[h] = pA.tile([128, TC], f32, name=f"k_ps{h}", tag=f"t{2 * h + 1}")
                            for kc in range(KC):
                                nc.tensor.matmul(k_ps[h][:], lhsT=wk_sb[:, kc, h * D:(h + 1) * D],
                                                 rhs=xtile(c, kc), start=(kc == 0), stop=(kc == KC - 1))
                            rope_drain(k_ps[h], kT[h], s0)
                    # part 2: V in natural layout (x-tile stationary); the
                    # next chunk's x stream is queued first so it prefetches
                    # during this chunk's remaining compute
                    if c + 1 < NCH:
                        fetch_x(c + 1)
                    for tb in range(TC // 128):
                        v_ps = pA.tile([128, DC], f32, name="v_ps", tag=f"t{2 * tb}")
                        for kc in range(KC):
                            nc.tensor.matmul(v_ps[:], lhsT=xtile(c, kc, tb * 128, (tb + 1) * 128),
                                             rhs=wv_sb[:, kc, :], start=(kc == 0), stop=(kc == KC - 1))
                        nc.vector.tensor_copy(v_sb[:, c * (TC // 128) + tb, :], v_ps[:])

            # ============ Phase B: causal attention (flat pipeline) ============
            # Score matmuls lead z/sum matmuls by a global LOOKAHEAD window
            # that flows ACROSS (head, superblock) unit boundaries, so the
            # exp pipeline never drains.  Phase C token-block groups are
            # interleaved after each superblock (they only need that
            # superblock's zn) and share the "st" PSUM tag, filling the
            # attention pipeline's dependency-wait slack with matmuls.
            with tc.tile_pool(name="psB", bufs=1, space="PSUM") as pB:
                tiles = []   # (sb, h, kblk)
                cpoint = {}
                for sb in range(NSB):
                    for h in range(HPC):
                        for kblk in range((sb + 1) * 4):
                            tiles.append((sb, h, kblk))
                    cpoint[len(tiles)] = sb
                pts = {}
                zsum = {}

                def emit_score(t):
                    sb, h, kblk = t
                    # diagonal tiles only need queries q >= 128*delta:
                    # restrict score/exp/mask to the live column range
                    delta = kblk - sb * 4
                    q0 = 128 * delta if delta > 0 else 0
                    st_ps = pB.tile([128, 512], f32, name="st_ps", tag="st", bufs=4)
                    nc.tensor.matmul(st_ps[:, q0:512],
                                     lhsT=kT[h][:, kblk * 128:(kblk + 1) * 128],
                                     rhs=qT[h][:, sb * 512 + q0:(sb + 1) * 512],
                                     start=True, stop=True)
                    pt = ptp.tile([128, 512], f16, name="pt", tag="pt")
                    nc.scalar.activation(pt[:, q0:512], st_ps[:, q0:512], Exp, scale=SCALE)
                    if delta >= 0:
                        # causal mask: multiply by a constant lower-triangle
                        # (DVE — lighter dispatch/sem path than gpsimd's
                        # affine_select in the exp->z critical chain)
                        nc.vector.tensor_tensor(pt[:, q0:q0 + 128], pt[:, q0:q0 + 128],
                                                tri_sb[:], mult)
                    pts[t] = (pt, q0)

                def emit_zsum(t):
                    sb, h, kblk = t
                    nkb = (sb + 1) * 4
                    if kblk == 0:
                        zsum[(sb, h)] = (
                            pB.tile([128, 512], f32, name="z_ps", tag="z", bufs=2),
                            pB.tile([128, 512], f32, name="sum_ps", tag="sum", bufs=2),
                        )
                    z_ps, sum_ps = zsum[(sb, h)]
                    pt, q0 = pts.pop(t)
                    nc.tensor.matmul(z_ps[:, q0:512], lhsT=v_sb[:, kblk, h * D:(h + 1) * D],
                                     rhs=pt[:, q0:512], start=(kblk == 0), stop=(kblk == nkb - 1))
                    nc.tensor.matmul(sum_ps[:, q0:512], lhsT=ones_sb[:],
                                     rhs=pt[:, q0:512], start=(kblk == 0), stop=(kblk == nkb - 1))
                    if kblk == nkb - 1:
                        rep_sb = smp.tile([128, 512], f32, name="rep_sb", tag="repsb")
                        nc.vector.reciprocal_approx_fast(rep_sb[:], sum_ps[:])
                        nc.vector.tensor_tensor(zn[h][:, sb * 512:(sb + 1) * 512],
                                                z_ps[:], rep_sb[:], mult)

                def emit_ctb(tb):
                    # output projection for one 128-token block: independent
                    # PE work that absorbs the attention pipeline's exp-wait
                    o_sb = op.tile([128, E], f16, name="o_sb", tag="osb")
                    for ec in range(E // 512):
                        o_ps = pB.tile([128, 512], f32, name="o_ps", tag="st", bufs=4)
                        for h in range(HPC):
                            nc.tensor.matmul(o_ps[:], lhsT=zn[h][:, tb * 128:(tb + 1) * 128],
                                             rhs=wo_sb[:, h, ec * 512:(ec + 1) * 512],
                                             start=(h == 0), stop=(h == HPC - 1))
                        if ec % 2 == 0:
                            nc.vector.tensor_copy(o_sb[:, ec * 512:(ec + 1) * 512], o_ps[:])
                        else:
                            nc.scalar.copy(o_sb[:, ec * 512:(ec + 1) * 512], o_ps[:])
                    nc.sync.dma_start(out=out_d[tb], in_=o_sb[:])

                for j in range(min(LOOKAHEAD, len(tiles))):
                    emit_score(tiles[j])
                for j, t in enumerate(tiles):
                    if j + LOOKAHEAD < len(tiles):
                        emit_score(tiles[j + LOOKAHEAD])
                    emit_zsum(t)
                    if (j + 1) in cpoint:
                        sb = cpoint[j + 1]
                        for tb in range(sb * 4, (sb + 1) * 4):
                            emit_ctb(tb)

    nc.compile()
    return nc


def _get_compiled():
    global _COMPILED
    if _COMPILED is None:
        _COMPILED = _build_program()
    return _COMPILED


def _host_inputs(x, wq, wk, wv, wo):
    x = np.asarray(x, dtype=np.float32)
    # per batch: xT blocked [KC, NCH, 128, TC]; (kc, c, p, t) = x[b, c*TC+t, kc*128+p]
    xTb = []
    for b in range(B):
        xb = np.ascontiguousarray(
            x[b].T.reshape(KC, 128, NCH, TC).transpose(0, 2, 1, 3).astype(np.float16)
        )
        xTb.append(xb)

    pos = np.arange(S, dtype=np.float32)
    inv_freq = (1.0 / (ROPE_BASE ** (np.arange(0, D, 2, dtype=np.float32) / np.float32(D)))).astype(np.float32)
    ang = pos[:, None] * inv_freq[None, :]          # (S, 64) fp32
    cos_h = np.cos(ang)
    sin_h = np.sin(ang)
    cosF = np.ascontiguousarray(np.concatenate([cos_h.T, cos_h.T], axis=0)).astype(np.float16)
    sinF = np.ascontiguousarray(np.concatenate([-sin_h.T, sin_h.T], axis=0)).astype(np.float16)
    ones = np.ones((128, 128), dtype=np.float16)
    # tri[p, j] = 1 where query j >= key p (within a diagonal 128-block)
    tri = np.tril(np.ones((128, 128), dtype=np.float16)).T.copy()

    wq = np.asarray(wq, dtype=np.float32)
    wk = np.asarray(wk, dtype=np.float32)
    wv = np.asarray(wv, dtype=np.float32)
    wo = np.asarray(wo, dtype=np.float32)

    maps = []
    for core in range(NCORES):
        b = core // (NCORES // BGRP)
        g = core % (NCORES // BGRP)
        sl = slice(g * DC, (g + 1) * DC)
        maps.append({
            "xT": xTb[b],
            "cosF": cosF,
            "sinF": sinF,
            "wq": np.ascontiguousarray(wq[:, sl].reshape(KC, 128, DC)).astype(np.float16),
            "wk": np.ascontiguousarray(wk[:, sl].reshape(KC, 128, DC)).astype(np.float16),
            "wv": np.ascontiguousarray(wv[:, sl].reshape(KC, 128, DC)).astype(np.float16),
            "wo": np.ascontiguousarray(wo[sl, :].reshape(HPC, 128, E).transpose(1, 0, 2)).astype(np.float16),
            "ones": ones,
            "tri": tri,
        })
    return maps


def kernel(x, wq, wk, wv, wo, _trace=False):
    from concourse.bass_utils import run_bass_kernel_spmd

    nc = _get_compiled()
    maps = _host_inputs(x, wq, wk, wv, wo)
    res = run_bass_kernel_spmd(nc, maps, list(range(NCORES)), trace=_trace)
    out = np.zeros((B, S, E), dtype=np.float32)
    for core in range(NCORES):
        b = core // (NCORES // BGRP)
        out[b] += res.results[core]["out"].astype(np.float32).reshape(S, E)
    if _trace:
        kernel.last_exec_time_ns = res.exec_time_ns
        kernel.last_trace = res.instructions_and_trace
    return out
